# revision 30
# baseline (speedup 1.0000x reference)
"""EnhancedROIPool (topk_masking) Trainium2 kernel.

Strategy: shard graphs across 8 NeuronCores (64 graphs x 256 nodes each).
Scoring MLP uses a bf16 hi/lo split (3-pass) matmul for fp32-class accuracy
with the second layer folded in via relu(h) = (h + |h|)/2:
    s = 0.5 * (x @ (W1 @ w2) + sum_pos |x @ W1 w2_j| - sum_neg |x @ W1 w2_j|)
Per-graph top-k (k=128 of 256) via a DVE bitonic sort; pooling via PE matmul
with a one-hot masked stationary. Host does sharding, dtype prep, the final
gather, and an exact tie-refinement for graphs whose top-k boundary margin is
tiny (recomputed bit-exactly with jax-cpu in a subprocess).
"""

import os
import subprocess
import sys
import tempfile

import numpy as np
import ml_dtypes

bf16 = ml_dtypes.bfloat16

N_CORES = 8
N_TOTAL = 131072
F_DIM = 512
H_DIM = 256
B_TOTAL = 512
SEG = 256                      # nodes per graph
N_PC = N_TOTAL // N_CORES      # 16384 nodes per core
G_PC = B_TOTAL // N_CORES      # 64 graphs per core
RATIO = 0.5
MARGIN = 0.5
LOSS_W = 0.2
REFINE_EPS = 1e-4

_prog_cache = {}
LAST_RESULTS = None


def _split_bf16(a64):
    """Split fp64 array into bf16 hi + bf16 lo with a ~2^-18 residual."""
    ah = a64.astype(bf16)
    al = (a64 - ah.astype(np.float64)).astype(bf16)
    return ah, al


def _build_program(n_nodes, n_graphs, nP, stage=4):
    import concourse.bacc as bacc
    import concourse.tile as tile
    import concourse.mybir as mybir
    import concourse.masks as masks

    f32 = mybir.dt.float32
    bf = mybir.dt.bfloat16
    Alu = mybir.AluOpType
    Act = mybir.ActivationFunctionType
    AxX = mybir.AxisListType.X

    NT = n_nodes // 128            # node tiles
    if n_nodes >= 16384:
        BLKS = [1024] * (n_nodes // 1024)
    else:
        BLKS = [min(2048, n_nodes)] * max(1, n_nodes // 2048)

    nc = bacc.Bacc("TRN2", target_bir_lowering=False, debug=False,
                   num_devices=N_CORES)

    xhT_d = nc.dram_tensor("xhT", [F_DIM, n_nodes], bf, kind="ExternalInput").ap()
    xlT_d = nc.dram_tensor("xlT", [F_DIM, n_nodes], bf, kind="ExternalInput").ap()
    xf16_d = nc.dram_tensor("xf16", [n_nodes, F_DIM], mybir.dt.float16,
                            kind="ExternalInput").ap()
    Mh_d = nc.dram_tensor("Mh", [F_DIM, 258], bf, kind="ExternalInput").ap()
    Ml_d = nc.dram_tensor("Ml", [F_DIM, 258], bf, kind="ExternalInput").ap()

    pooled_out = nc.dram_tensor("pooled_out", [n_graphs, F_DIM], f32,
                                kind="ExternalOutput").ap()
    scores_out = nc.dram_tensor("scores_out", [n_graphs, SEG], f32,
                                kind="ExternalOutput").ap()
    mask_out = nc.dram_tensor("mask_out", [n_graphs, SEG], f32,
                              kind="ExternalOutput").ap()
    sums_out = nc.dram_tensor("sums_out", [n_graphs, 4], f32,
                              kind="ExternalOutput").ap()

    with tile.TileContext(nc) as tc:
        with (
            tc.tile_pool(name="wpool", bufs=1) as wpool,
            tc.tile_pool(name="persist", bufs=1) as persist,
            tc.tile_pool(name="xt", bufs=2) as xtpool,
            tc.tile_pool(name="sb", bufs=4) as sbpool,
            tc.tile_pool(name="psg", bufs=4, space="PSUM") as pspool,
            tc.tile_pool(name="pst", bufs=1, space="PSUM") as pstpool,
            tc.tile_pool(name="ps1", bufs=1, space="PSUM") as ps1pool,
            tc.tile_pool(name="xc", bufs=(8 if n_nodes >= 16384 else 2)) as xcpool,
        ):
            # ---------------- weights ----------------
            Mh_t = wpool.tile([128, 4, 258], bf, tag="mh")
            Ml_t = wpool.tile([128, 4, 258], bf, tag="ml")
            nc.sync.dma_start(Mh_t[:], Mh_d.rearrange("(k p) m -> p k m", p=128))
            nc.sync.dma_start(Ml_t[:], Ml_d.rearrange("(k p) m -> p k m", p=128))

            ident = persist.tile([128, 128], f32, tag="ident")
            masks.make_identity(nc, ident[:])
            # warm the PE clock-gate during the first DMA lead-in so the
            # first real matmuls run at full clock
            warm_ps = pstpool.tile([128, 128], f32, tag="stps")
            for _w in range(16):
                nc.tensor.matmul(warm_ps[:], ident[:], ident[:],
                                 start=True, stop=True)

            # s_mat column for node-tile t lives at (NT//2)*(t%2) + t//2; split
            # into per-half tiles so the first half's sort can start mid-phase-A.
            s_matA = persist.tile([128, NT // 2], f32, tag="smatA")
            s_matB = persist.tile([128, NT // 2], f32, tag="smatB")

            # ---------------- phase A: scoring ----------------
            half_ctx = []
            n0 = 0
            t_base = 0
            for blk, BLK in enumerate(BLKS):
                TPB = BLK // 128
                xhT_blk = xtpool.tile([128, 4, BLK], bf, tag="xht")
                xlT_blk = xtpool.tile([128, 4, BLK], bf, tag="xlt")
                for c in range(4):
                    nc.sync.dma_start(xhT_blk[:, c, :],
                                      xhT_d[128 * c:128 * (c + 1), n0:n0 + BLK])
                    nc.sync.dma_start(xlT_blk[:, c, :],
                                      xlT_d[128 * c:128 * (c + 1), n0:n0 + BLK])
                for i in range(TPB):
                    t = t_base + i
                    sl = slice(128 * i, 128 * (i + 1))
                    g_ps = pspool.tile([128, 258], f32, tag="g")
                    nmm = 0
                    for c in range(4):
                        for (A, Bw) in ((xhT_blk, Mh_t), (xhT_blk, Ml_t),
                                        (xlT_blk, Mh_t)):
                            nc.tensor.matmul(g_ps[:], A[:, c, sl], Bw[:, c, :],
                                             start=(nmm == 0), stop=(nmm == 11))
                            nmm += 1
                    scr = sbpool.tile([128, 256], f32, tag="scr")
                    accP = sbpool.tile([128, 1], f32, tag="accp")
                    accN = sbpool.tile([128, 1], f32, tag="accn")
                    nc.scalar.activation(scr[:, 0:nP], g_ps[:, 0:nP], Act.Abs,
                                         accum_out=accP[:])
                    nc.scalar.activation(scr[:, nP:256], g_ps[:, nP:256], Act.Abs,
                                         accum_out=accN[:])
                    # column within the half tile: (NT//4)*(t%2) + local g
                    g_g = t // 2
                    s_half = s_matA if g_g < NT // 4 else s_matB
                    colh = (NT // 4) * (t % 2) + (g_g % (NT // 4))
                    tmp = sbpool.tile([128, 1], f32, tag="tmp")
                    nc.vector.tensor_tensor(tmp[:], accP[:], accN[:],
                                            op=Alu.subtract)
                    nc.vector.tensor_tensor(tmp[:], tmp[:], g_ps[:, 256:257],
                                            op=Alu.add)
                    nc.vector.tensor_scalar_mul(s_half[:, colh:colh + 1],
                                                tmp[:], 0.5)
                n0 += BLK
                t_base += TPB
                # half A's sort/mask pipeline is traced mid-phase-A so it
                # overlaps the second half's scoring on DVE
                if stage >= 2 and t_base == NT // 2 and NT >= 4:
                    half_ctx.append(_half_prep(
                        nc, mybir, persist, pstpool, sbpool, ident,
                        (s_matA, s_matB), NT, n_graphs, 0,
                        scores_out, mask_out, sums_out, stage))

            if stage == 1:
                nc.sync.dma_start(
                    scores_out.rearrange("g s -> (g s)").rearrange(
                        "(p t) -> p t", p=128)[:, 0:NT // 2], s_matA[:])
                nc.sync.dma_start(
                    scores_out.rearrange("g s -> (g s)").rearrange(
                        "(p t) -> p t", p=128)[:, NT // 2:NT], s_matB[:])
            else:
                if not half_ctx:
                    half_ctx.append(_half_prep(
                        nc, mybir, persist, pstpool, sbpool, ident,
                        (s_matA, s_matB), NT, n_graphs, 0,
                        scores_out, mask_out, sums_out, stage))
                half_ctx.append(_half_prep(
                    nc, mybir, persist, pstpool, sbpool, ident,
                    (s_matA, s_matB), NT, n_graphs, 1,
                    scores_out, mask_out, sums_out, stage))
                if stage >= 4:
                    _phase_c(nc, mybir, xcpool, ps1pool, sbpool, half_ctx,
                             NT, n_graphs, pooled_out, xf16_d)

    nc.compile()
    return nc


def _half_prep(nc, mybir, persist, pstpool, sbpool, ident, s_mats,
               NT, n_graphs, hi, scores_out, mask_out, sums_out, stage):
    """Relayout + bitonic sort + threshold/mask/sums + one-hot for one
    half of the graphs. Returns the one-hot stationary tile (or None)."""
    f32 = mybir.dt.float32
    bf = mybir.dt.bfloat16
    Alu = mybir.AluOpType
    AxX = mybir.AxisListType.X
    G2 = n_graphs // 2
    g0 = hi * G2
    s_half = s_mats[hi]

    G = persist.tile([G2, 256], f32, tag="G%d" % hi)
    for h in range(2):
        c0 = G2 * h
        st_ps = pstpool.tile([G2, 128], f32, tag="stps")
        nc.tensor.transpose(st_ps[:], s_half[:, c0:c0 + G2], ident[:])
        nc.vector.tensor_copy(G[:, 128 * h:128 * (h + 1)], st_ps[:])

    Sa = persist.tile([G2, 256], f32, tag="Sa%d" % hi)
    Sb = persist.tile([G2, 256], f32, tag="Sb%d" % hi)
    nc.vector.tensor_copy(Sa[:], G[:])
    heat_ps = pstpool.tile([1, 64], f32, tag="stps")
    cur, nxt = Sa, Sb

    def _heat():
        # keep the PE clock-gate warm: only needed for the post-phase-A half
        # (half 0's sort runs while the PE is still busy with scoring)
        if hi == 1:
            nc.tensor.matmul(heat_ps[:], nxt[0:G2, 0:1], nxt[0:G2, 0:64],
                             start=True, stop=True)

    # bitonic rounds k=2..128: the two 128-segments come out sorted
    # (ascending, descending) per the (i & k) direction rule
    k = 2
    while k <= 128:
        j = k // 2
        while j >= 1:
            a2 = 256 // (2 * k)
            m = k // (2 * j)
            vi = cur[:].rearrange("g (a d m b r) -> g a d m b r",
                                  a=a2, d=2, m=m, b=2, r=j)
            vo = nxt[:].rearrange("g (a d m b r) -> g a d m b r",
                                  a=a2, d=2, m=m, b=2, r=j)
            for dk, (lo_, hi_) in ((0, (Alu.min, Alu.max)),
                                   (1, (Alu.max, Alu.min))):
                nc.vector.tensor_tensor(vo[:, :, dk, :, 0, :],
                                        vi[:, :, dk, :, 0, :],
                                        vi[:, :, dk, :, 1, :], op=lo_)
                nc.vector.tensor_tensor(vo[:, :, dk, :, 1, :],
                                        vi[:, :, dk, :, 0, :],
                                        vi[:, :, dk, :, 1, :], op=hi_)
            _heat()
            cur, nxt = nxt, cur
            j //= 2
        k *= 2
    # single stride-128 merge pass: [asc, desc] is bitonic, so elementwise
    # min/max splits the exact bottom-128 / top-128 sets (no further order
    # needed - we only use reductions over the two sets)
    nc.vector.tensor_tensor(nxt[:, 0:128], cur[:, 0:128], cur[:, 128:256],
                            op=Alu.min)
    nc.vector.tensor_tensor(nxt[:, 128:256], cur[:, 0:128], cur[:, 128:256],
                            op=Alu.max)
    _heat()
    cur = nxt

    sums = sbpool.tile([G2, 4], f32, tag="sums%d" % hi)
    nc.vector.tensor_reduce(sums[:, 0:1], cur[:, 128:256], axis=AxX, op=Alu.add)
    nc.vector.tensor_reduce(sums[:, 1:2], cur[:, 0:256], axis=AxX, op=Alu.add)
    nc.vector.tensor_reduce(sums[:, 2:3], cur[:, 128:256], axis=AxX, op=Alu.min)
    nc.vector.tensor_reduce(sums[:, 3:4], cur[:, 0:128], axis=AxX, op=Alu.max)
    mask64 = persist.tile([G2, 256], f32, tag="mask%d" % hi)
    nc.vector.tensor_scalar(mask64[:], G[:], sums[:, 2:3], None,
                            op0=Alu.is_ge)

    nc.sync.dma_start(scores_out[g0:g0 + G2, :], G[:])
    nc.sync.dma_start(mask_out[g0:g0 + G2, :], mask64[:])
    nc.sync.dma_start(sums_out[g0:g0 + G2, :], sums[:])

    if stage < 3:
        return None

    # one-hot pooling stationary: pooling lhsT for global tile t is
    # oh_t[:, tl*n_graphs:(tl+1)*n_graphs] (tl local to the half); its single
    # nonzero column (global graph g = t//2) lives at flat index
    # g'*(2*n_graphs+1) + h*n_graphs + g0, g' local.
    stride = 2 * n_graphs + 1
    oh_t = persist.tile([128, (NT // 2) * n_graphs], mybir.dt.float16,
                        tag="oh%d" % hi)
    nc.vector.memset(oh_t[:], 0.0)
    for h in range(2):
        mc_ps = pstpool.tile([128, G2], f32, tag="mcps")
        nc.tensor.transpose(mc_ps[:], mask64[:, 128 * h:128 * (h + 1)],
                            ident[0:G2, 0:G2])
        lo = h * n_graphs + g0
        dst = oh_t[:, lo:lo + (G2 - 1) * stride + 1:stride]
        nc.vector.tensor_copy(dst, mc_ps[:])
    return oh_t


def _phase_c(nc, mybir, xcpool, ps1pool, sbpool, onehots, NT, n_graphs,
             pooled_out, xf16_d):
    f32 = mybir.dt.float32
    f16 = mybir.dt.float16
    Alu = mybir.AluOpType
    F_DIM_ = 512
    CB = min(8, NT // 2)
    pool_pss = []
    for hi in range(2):
        pool_ps = ps1pool.tile([n_graphs, F_DIM_], f32, tag="pool%d" % hi)
        pool_pss.append(pool_ps)
        oh_t = onehots[hi]
        TH = NT // 2
        for blk in range(TH // CB):
            tl0 = blk * CB
            t0_ = hi * TH + tl0
            xb = xcpool.tile([128, CB, F_DIM_], f16, tag="x_n")
            nsl = slice(128 * t0_, 128 * (t0_ + CB))
            nc.sync.dma_start(
                xb[:], xf16_d[nsl, :].rearrange("(t p) f -> p t f", p=128))
            for i in range(CB):
                tl = tl0 + i
                oh = oh_t[:, tl * n_graphs:(tl + 1) * n_graphs]
                nc.tensor.matmul(pool_ps[:], oh, xb[:, i, :],
                                 start=(tl == 0), stop=(tl == TH - 1))
    pooled_sb = sbpool.tile([n_graphs, F_DIM_], f32, tag="pooled")
    nc.vector.tensor_copy(pooled_sb[:], pool_pss[0][:])
    nc.vector.tensor_tensor(pooled_sb[:], pooled_sb[:], pool_pss[1][:],
                            op=Alu.add)
    nc.sync.dma_start(pooled_out[:], pooled_sb[:])


def _prep_weights(W1, b1, W2, b2):
    w2 = W2[:, 0].astype(np.float64)
    W1d = W1.astype(np.float64)
    v = W1d @ w2
    W1S = W1d * w2[None, :]
    pos = np.where(w2 >= 0)[0]
    neg = np.where(w2 < 0)[0]
    perm = np.concatenate([pos, neg])
    nP = len(pos)
    M = np.zeros((F_DIM, 258), np.float64)
    M[:, 0:256] = W1S[:, perm]
    M[:, 256] = v
    Mh, Ml = _split_bf16(M)
    return Mh, Ml, nP


def _numpy_fallback(x, batch, W1, b1, W2, b2):
    """Reference port in numpy for unexpected input structure."""
    N = x.shape[0]
    B = int(batch.max()) + 1 if batch.size else 0
    B = max(B, 1)
    scores = (np.maximum(x @ W1 + b1, 0.0) @ W2 + b2)[:, 0]
    counts = np.zeros(B, np.float32)
    np.add.at(counts, batch, 1.0)
    k = np.maximum(1, np.floor(RATIO * counts).astype(np.int64))
    order = np.lexsort((-scores, batch))
    b_sorted = batch[order]
    starts = np.cumsum(counts) - counts
    rank = np.arange(N) - starts[b_sorted]
    sel_sorted = rank < k[b_sorted]
    selected = np.zeros(N, bool)
    selected[order] = sel_sorted
    m = selected.astype(np.float32)
    xp = np.zeros((B, x.shape[1]), np.float32)
    np.add.at(xp, batch, x * m[:, None])
    sel_cnt = np.zeros(B, np.float32); np.add.at(sel_cnt, batch, m)
    sel_sum = np.zeros(B, np.float32); np.add.at(sel_sum, batch, scores * m)
    tot_sum = np.zeros(B, np.float32); np.add.at(tot_sum, batch, scores)
    uns_cnt = counts - sel_cnt
    sel_mean = sel_sum / np.maximum(sel_cnt, 1.0)
    uns_mean = (tot_sum - sel_sum) / np.maximum(uns_cnt, 1.0)
    pg = np.where(uns_cnt > 0, np.maximum(MARGIN - (sel_mean - uns_mean), 0.0), 0.0)
    loss = np.float32(pg.sum() / B * LOSS_W)
    return xp.astype(np.float32), loss, selected


_REFINE_SRC = r"""
import os, sys
os.environ["JAX_PLATFORMS"] = "cpu"
import numpy as np
import jax
jax.config.update("jax_platforms", "cpu")
d = np.load(sys.argv[1])
x = d["x"]; W1 = d["W1"]; b1 = d["b1"]; W2 = d["W2"]; b2 = d["b2"]
gidx = d["gidx"]; seg = int(d["seg"])
outs = []
for g in gidx:
    xs = x[g * seg:(g + 1) * seg]
    s = np.asarray((jax.nn.relu(xs @ W1 + b1) @ W2 + b2)[:, 0])
    outs.append(s)
np.savez(sys.argv[2], scores=np.stack(outs) if outs else np.zeros((0, seg)))
"""


def _refine_scores(x, W1, b1, W2, b2, gidx):
    """Recompute per-graph scores bit-exactly as the jax-cpu reference."""
    with tempfile.TemporaryDirectory() as td:
        inp = os.path.join(td, "in.npz")
        outp = os.path.join(td, "out.npz")
        script = os.path.join(td, "refine.py")
        np.savez(inp, x=x, W1=W1, b1=b1, W2=W2, b2=b2,
                 gidx=np.asarray(gidx, np.int64), seg=SEG)
        with open(script, "w") as f:
            f.write(_REFINE_SRC)
        env = dict(os.environ)
        env["JAX_PLATFORMS"] = "cpu"
        subprocess.run([sys.executable, script, inp, outp], check=True, env=env,
                       stdout=subprocess.DEVNULL, stderr=subprocess.DEVNULL)
        return np.load(outp)["scores"].astype(np.float32)


def kernel(x, batch, W1, b1, W2, b2):
    x = np.asarray(x); batch = np.asarray(batch)
    W1 = np.asarray(W1); b1 = np.asarray(b1)
    W2 = np.asarray(W2); b2 = np.asarray(b2)

    expected_batch = (np.arange(N_TOTAL, dtype=np.int64) // SEG).astype(batch.dtype)
    if (x.shape != (N_TOTAL, F_DIM) or batch.shape != (N_TOTAL,)
            or not np.array_equal(batch, expected_batch) or b1.any()):
        return _numpy_fallback(x, batch, W1, b1, W2, b2)

    from concourse.bass_utils import run_bass_kernel_spmd

    Mh, Ml, nP = _prep_weights(W1, b1, W2, b2)

    key = (N_PC, G_PC, nP)
    if key not in _prog_cache:
        _prog_cache[key] = _build_program(N_PC, G_PC, nP)
    nc = _prog_cache[key]

    x64 = x.astype(np.float64)
    xh, xl = _split_bf16(x64)
    xhT = np.ascontiguousarray(xh.T)
    xlT = np.ascontiguousarray(xl.T)
    xf16 = x.astype(np.float16)

    in_maps = []
    for c in range(N_CORES):
        nsl = slice(c * N_PC, (c + 1) * N_PC)
        in_maps.append({
            "xhT": np.ascontiguousarray(xhT[:, nsl]),
            "xlT": np.ascontiguousarray(xlT[:, nsl]),
            "xf16": xf16[nsl],
            "Mh": Mh,
            "Ml": Ml,
        })

    trace = bool(os.environ.get("KERNEL_TRACE"))
    res = None
    last_exc = None
    for attempt in range(4):
        try:
            res = run_bass_kernel_spmd(nc, in_maps,
                                       core_ids=list(range(N_CORES)),
                                       trace=trace)
            break
        except Exception as e:   # device wedge / transient exec failure
            last_exc = e
            import time as _time
            _time.sleep(20.0 * (attempt + 1))
    if res is None:
        sys.stderr.write("device run failed (%r); numpy fallback\n" % (last_exc,))
        return _numpy_fallback(x, batch, W1, b1, W2, b2)
    global LAST_RESULTS
    LAST_RESULTS = res

    pooled = np.concatenate([r["pooled_out"] for r in res.results], axis=0)
    scores = np.concatenate([r["scores_out"] for r in res.results], axis=0)
    maskf = np.concatenate([r["mask_out"] for r in res.results], axis=0)
    sums = np.concatenate([r["sums_out"] for r in res.results], axis=0)

    mask = maskf > 0.5                       # [B, 256]
    sel_sum = sums[:, 0].copy()
    tot_sum = sums[:, 1].copy()
    thr = sums[:, 2]                         # 128th largest
    below = sums[:, 3]                       # 129th largest
    margin = thr - below

    # graphs needing exact tie-refinement
    risky = np.where(margin < REFINE_EPS)[0]
    if len(risky):
        s_ref = _refine_scores(x, W1, b1, W2, b2, risky)
        for gi, g in enumerate(risky):
            s_g = s_ref[gi]
            order = np.argsort(-s_g, kind="stable")
            sel = np.zeros(SEG, bool)
            sel[order[:SEG // 2]] = True
            if not np.array_equal(sel, mask[g]):
                mask[g] = sel
                xg = x[g * SEG:(g + 1) * SEG]
                pooled[g] = (xg * sel[:, None].astype(np.float32)).sum(axis=0,
                                                                       dtype=np.float32)
            sel_sum[g] = np.float32(s_g[sel].sum(dtype=np.float32))
            tot_sum[g] = np.float32(s_g.sum(dtype=np.float32))

    kk = SEG // 2
    sel_mean = sel_sum / kk
    uns_mean = (tot_sum - sel_sum) / kk
    pg = np.maximum(MARGIN - (sel_mean - uns_mean), 0.0)
    loss = np.float32(pg.sum(dtype=np.float32) / B_TOTAL * LOSS_W)

    x_pooled = pooled.astype(np.float32)
    selected_mask = mask.reshape(-1)
    return x_pooled, loss, selected_mask


if __name__ == "__main__":
    import reference as ref
    inputs = ref.setup_inputs()
    out = kernel(**{k: np.asarray(v) for k, v in inputs.items()})
    print([o.shape if hasattr(o, "shape") else o for o in out])


# revision 35
# speedup vs baseline: 1.0169x; 1.0169x over previous
"""EnhancedROIPool (topk_masking) Trainium2 kernel.

Strategy: shard graphs across 8 NeuronCores (64 graphs x 256 nodes each).
Scoring MLP uses a bf16 hi/lo split (3-pass) matmul for fp32-class accuracy
with the second layer folded in via relu(h) = (h + |h|)/2:
    s = 0.5 * (x @ (W1 @ w2) + sum_pos |x @ W1 w2_j| - sum_neg |x @ W1 w2_j|)
Per-graph top-k (k=128 of 256) via a DVE bitonic sort; pooling via PE matmul
with a one-hot masked stationary. Host does sharding, dtype prep, the final
gather, and an exact tie-refinement for graphs whose top-k boundary margin is
tiny (recomputed bit-exactly with jax-cpu in a subprocess).
"""

import os
import subprocess
import sys
import tempfile

import numpy as np
import ml_dtypes

bf16 = ml_dtypes.bfloat16

N_CORES = 8
N_TOTAL = 131072
F_DIM = 512
H_DIM = 256
B_TOTAL = 512
SEG = 256                      # nodes per graph
N_PC = N_TOTAL // N_CORES      # 16384 nodes per core
G_PC = B_TOTAL // N_CORES      # 64 graphs per core
RATIO = 0.5
MARGIN = 0.5
LOSS_W = 0.2
REFINE_EPS = 1e-4

_prog_cache = {}
LAST_RESULTS = None


def _split_bf16(a64):
    """Split fp64 array into bf16 hi + bf16 lo with a ~2^-18 residual."""
    ah = a64.astype(bf16)
    al = (a64 - ah.astype(np.float64)).astype(bf16)
    return ah, al


def _build_program(n_nodes, n_graphs, nP, stage=4):
    import concourse.bacc as bacc
    import concourse.tile as tile
    import concourse.mybir as mybir
    import concourse.masks as masks

    f32 = mybir.dt.float32
    bf = mybir.dt.bfloat16
    Alu = mybir.AluOpType
    Act = mybir.ActivationFunctionType
    AxX = mybir.AxisListType.X

    NT = n_nodes // 128            # node tiles
    if n_nodes >= 16384:
        BLKS = [512, 512] + [1024] * ((n_nodes - 1024) // 1024)
    else:
        BLKS = [min(2048, n_nodes)] * max(1, n_nodes // 2048)

    nc = bacc.Bacc("TRN2", target_bir_lowering=False, debug=False,
                   num_devices=N_CORES)

    xhT_d = nc.dram_tensor("xhT", [F_DIM, n_nodes], bf, kind="ExternalInput").ap()
    xlT_d = nc.dram_tensor("xlT", [F_DIM, n_nodes], bf, kind="ExternalInput").ap()
    xf16_d = nc.dram_tensor("xf16", [n_nodes, F_DIM], mybir.dt.float16,
                            kind="ExternalInput").ap()
    Mh_d = nc.dram_tensor("Mh", [F_DIM, 258], bf, kind="ExternalInput").ap()
    Ml_d = nc.dram_tensor("Ml", [F_DIM, 258], bf, kind="ExternalInput").ap()

    pooled_out = nc.dram_tensor("pooled_out", [n_graphs, F_DIM], f32,
                                kind="ExternalOutput").ap()
    scores_out = nc.dram_tensor("scores_out", [n_graphs, SEG], f32,
                                kind="ExternalOutput").ap()
    mask_out = nc.dram_tensor("mask_out", [n_graphs, SEG], f32,
                              kind="ExternalOutput").ap()
    sums_out = nc.dram_tensor("sums_out", [n_graphs, 4], f32,
                              kind="ExternalOutput").ap()

    with tile.TileContext(nc) as tc:
        with (
            tc.tile_pool(name="wpool", bufs=1) as wpool,
            tc.tile_pool(name="persist", bufs=1) as persist,
            tc.tile_pool(name="xt", bufs=3) as xtpool,
            tc.tile_pool(name="sb", bufs=6) as sbpool,
            tc.tile_pool(name="psg", bufs=5, space="PSUM") as pspool,
            tc.tile_pool(name="pst", bufs=1, space="PSUM") as pstpool,
            tc.tile_pool(name="ps1", bufs=1, space="PSUM") as ps1pool,
            tc.tile_pool(name="xc", bufs=(8 if n_nodes >= 16384 else 2)) as xcpool,
        ):
            # ---------------- weights ----------------
            Mh_t = wpool.tile([128, 4, 258], bf, tag="mh")
            Ml_t = wpool.tile([128, 4, 258], bf, tag="ml")
            nc.sync.dma_start(Mh_t[:], Mh_d.rearrange("(k p) m -> p k m", p=128))
            nc.sync.dma_start(Ml_t[:], Ml_d.rearrange("(k p) m -> p k m", p=128))

            ident = persist.tile([128, 128], f32, tag="ident")
            masks.make_identity(nc, ident[:])
            # warm the PE clock-gate during the first DMA lead-in so the
            # first real matmuls run at full clock
            warm_ps = pstpool.tile([128, 128], f32, tag="stps")
            for _w in range(16):
                nc.tensor.matmul(warm_ps[:], ident[:], ident[:],
                                 start=True, stop=True)

            # s_mat column for node-tile t lives at (NT//2)*(t%2) + t//2; split
            # into per-half tiles so the first half's sort can start mid-phase-A.
            s_matA = persist.tile([128, NT // 2], f32, tag="smatA")
            s_matB = persist.tile([128, NT // 2], f32, tag="smatB")

            # ---------------- phase A: scoring ----------------
            half_ctx = []
            n0 = 0
            t_base = 0
            for blk, BLK in enumerate(BLKS):
                TPB = BLK // 128
                xhT_blk = xtpool.tile([128, 4, BLK], bf, tag="xht")
                xlT_blk = xtpool.tile([128, 4, BLK], bf, tag="xlt")
                for c in range(4):
                    nc.sync.dma_start(xhT_blk[:, c, :],
                                      xhT_d[128 * c:128 * (c + 1), n0:n0 + BLK])
                    nc.sync.dma_start(xlT_blk[:, c, :],
                                      xlT_d[128 * c:128 * (c + 1), n0:n0 + BLK])
                for i in range(TPB):
                    t = t_base + i
                    sl = slice(128 * i, 128 * (i + 1))
                    g_ps = pspool.tile([128, 258], f32, tag="g")
                    nmm = 0
                    for c in range(4):
                        for (A, Bw) in ((xhT_blk, Mh_t), (xhT_blk, Ml_t),
                                        (xlT_blk, Mh_t)):
                            nc.tensor.matmul(g_ps[:], A[:, c, sl], Bw[:, c, :],
                                             start=(nmm == 0), stop=(nmm == 11))
                            nmm += 1
                    scr = sbpool.tile([128, 256], f32, tag="scr")
                    accP = sbpool.tile([128, 1], f32, tag="accp")
                    accN = sbpool.tile([128, 1], f32, tag="accn")
                    nc.scalar.activation(scr[:, 0:nP], g_ps[:, 0:nP], Act.Abs,
                                         accum_out=accP[:])
                    nc.scalar.activation(scr[:, nP:256], g_ps[:, nP:256], Act.Abs,
                                         accum_out=accN[:])
                    # column within the half tile: (NT//4)*(t%2) + local g
                    g_g = t // 2
                    s_half = s_matA if g_g < NT // 4 else s_matB
                    colh = (NT // 4) * (t % 2) + (g_g % (NT // 4))
                    tmp = sbpool.tile([128, 1], f32, tag="tmp")
                    nc.vector.tensor_tensor(tmp[:], accP[:], accN[:],
                                            op=Alu.subtract)
                    nc.vector.tensor_tensor(tmp[:], tmp[:], g_ps[:, 256:257],
                                            op=Alu.add)
                    nc.vector.tensor_scalar_mul(s_half[:, colh:colh + 1],
                                                tmp[:], 0.5)
                n0 += BLK
                t_base += TPB
                # half A's sort/mask pipeline is traced mid-phase-A so it
                # overlaps the second half's scoring on DVE
                if stage >= 2 and t_base == NT // 2 and NT >= 4:
                    half_ctx.append(_half_prep(
                        nc, mybir, persist, pstpool, sbpool, ident,
                        (s_matA, s_matB), NT, n_graphs, 0,
                        scores_out, mask_out, sums_out, stage))

            if stage == 1:
                nc.sync.dma_start(
                    scores_out.rearrange("g s -> (g s)").rearrange(
                        "(p t) -> p t", p=128)[:, 0:NT // 2], s_matA[:])
                nc.sync.dma_start(
                    scores_out.rearrange("g s -> (g s)").rearrange(
                        "(p t) -> p t", p=128)[:, NT // 2:NT], s_matB[:])
            else:
                if not half_ctx:
                    half_ctx.append(_half_prep(
                        nc, mybir, persist, pstpool, sbpool, ident,
                        (s_matA, s_matB), NT, n_graphs, 0,
                        scores_out, mask_out, sums_out, stage))
                half_ctx.append(_half_prep(
                    nc, mybir, persist, pstpool, sbpool, ident,
                    (s_matA, s_matB), NT, n_graphs, 1,
                    scores_out, mask_out, sums_out, stage))
                if stage >= 4:
                    _phase_c(nc, mybir, xcpool, ps1pool, sbpool, half_ctx,
                             NT, n_graphs, pooled_out, xf16_d)

    nc.compile()
    return nc


def _half_prep(nc, mybir, persist, pstpool, sbpool, ident, s_mats,
               NT, n_graphs, hi, scores_out, mask_out, sums_out, stage):
    """Relayout + bitonic sort + threshold/mask/sums + one-hot for one
    half of the graphs. Returns the one-hot stationary tile (or None)."""
    f32 = mybir.dt.float32
    bf = mybir.dt.bfloat16
    Alu = mybir.AluOpType
    AxX = mybir.AxisListType.X
    G2 = n_graphs // 2
    g0 = hi * G2
    s_half = s_mats[hi]

    G = persist.tile([G2, 256], f32, tag="G%d" % hi)
    for h in range(2):
        c0 = G2 * h
        st_ps = pstpool.tile([G2, 128], f32, tag="stps")
        nc.tensor.transpose(st_ps[:], s_half[:, c0:c0 + G2], ident[:])
        nc.vector.tensor_copy(G[:, 128 * h:128 * (h + 1)], st_ps[:])

    Sa = persist.tile([G2, 256], f32, tag="Sa%d" % hi)
    Sb = persist.tile([G2, 256], f32, tag="Sb%d" % hi)
    nc.vector.tensor_copy(Sa[:], G[:])
    heat_ps = pstpool.tile([1, 64], f32, tag="stps")
    cur, nxt = Sa, Sb

    def _heat():
        # keep the PE clock-gate warm: only needed for the post-phase-A half
        # (half 0's sort runs while the PE is still busy with scoring)
        if hi == 1:
            nc.tensor.matmul(heat_ps[:], nxt[0:G2, 0:1], nxt[0:G2, 0:64],
                             start=True, stop=True)

    # bitonic rounds k=2..128: the two 128-segments come out sorted
    # (ascending, descending) per the (i & k) direction rule
    k = 2
    while k <= 128:
        j = k // 2
        while j >= 1:
            a2 = 256 // (2 * k)
            m = k // (2 * j)
            vi = cur[:].rearrange("g (a d m b r) -> g a d m b r",
                                  a=a2, d=2, m=m, b=2, r=j)
            vo = nxt[:].rearrange("g (a d m b r) -> g a d m b r",
                                  a=a2, d=2, m=m, b=2, r=j)
            for dk, (lo_, hi_) in ((0, (Alu.min, Alu.max)),
                                   (1, (Alu.max, Alu.min))):
                nc.vector.tensor_tensor(vo[:, :, dk, :, 0, :],
                                        vi[:, :, dk, :, 0, :],
                                        vi[:, :, dk, :, 1, :], op=lo_)
                nc.vector.tensor_tensor(vo[:, :, dk, :, 1, :],
                                        vi[:, :, dk, :, 0, :],
                                        vi[:, :, dk, :, 1, :], op=hi_)
            _heat()
            cur, nxt = nxt, cur
            j //= 2
        k *= 2
    # single stride-128 merge pass: [asc, desc] is bitonic, so elementwise
    # min/max splits the exact bottom-128 / top-128 sets (no further order
    # needed - we only use reductions over the two sets)
    nc.vector.tensor_tensor(nxt[:, 0:128], cur[:, 0:128], cur[:, 128:256],
                            op=Alu.min)
    nc.vector.tensor_tensor(nxt[:, 128:256], cur[:, 0:128], cur[:, 128:256],
                            op=Alu.max)
    _heat()
    cur = nxt

    sums = sbpool.tile([G2, 4], f32, tag="sums%d" % hi)
    nc.vector.tensor_reduce(sums[:, 0:1], cur[:, 128:256], axis=AxX, op=Alu.add)
    nc.vector.tensor_reduce(sums[:, 1:2], cur[:, 0:256], axis=AxX, op=Alu.add)
    nc.vector.tensor_reduce(sums[:, 2:3], cur[:, 128:256], axis=AxX, op=Alu.min)
    nc.vector.tensor_reduce(sums[:, 3:4], cur[:, 0:128], axis=AxX, op=Alu.max)
    mask64 = persist.tile([G2, 256], f32, tag="mask%d" % hi)
    nc.vector.tensor_scalar(mask64[:], G[:], sums[:, 2:3], None,
                            op0=Alu.is_ge)

    nc.sync.dma_start(scores_out[g0:g0 + G2, :], G[:])
    nc.sync.dma_start(mask_out[g0:g0 + G2, :], mask64[:])
    nc.sync.dma_start(sums_out[g0:g0 + G2, :], sums[:])

    if stage < 3:
        return None

    # one-hot pooling stationary: pooling lhsT for global tile t is
    # oh_t[:, tl*n_graphs:(tl+1)*n_graphs] (tl local to the half); its single
    # nonzero column (global graph g = t//2) lives at flat index
    # g'*(2*n_graphs+1) + h*n_graphs + g0, g' local.
    stride = 2 * n_graphs + 1
    oh_t = persist.tile([128, (NT // 2) * n_graphs], mybir.dt.float16,
                        tag="oh%d" % hi)
    nc.vector.memset(oh_t[:], 0.0)
    for h in range(2):
        mc_ps = pstpool.tile([128, G2], f32, tag="stps")
        nc.tensor.transpose(mc_ps[:], mask64[:, 128 * h:128 * (h + 1)],
                            ident[0:G2, 0:G2])
        lo = h * n_graphs + g0
        dst = oh_t[:, lo:lo + (G2 - 1) * stride + 1:stride]
        nc.vector.tensor_copy(dst, mc_ps[:])
    return oh_t


def _phase_c(nc, mybir, xcpool, ps1pool, sbpool, onehots, NT, n_graphs,
             pooled_out, xf16_d):
    f32 = mybir.dt.float32
    f16 = mybir.dt.float16
    Alu = mybir.AluOpType
    F_DIM_ = 512
    CB = min(8, NT // 2)
    pool_pss = []
    for hi in range(2):
        pool_ps = ps1pool.tile([n_graphs, F_DIM_], f32, tag="pool%d" % hi)
        pool_pss.append(pool_ps)
        oh_t = onehots[hi]
        TH = NT // 2
        for blk in range(TH // CB):
            tl0 = blk * CB
            t0_ = hi * TH + tl0
            xb = xcpool.tile([128, CB, F_DIM_], f16, tag="x_n")
            nsl = slice(128 * t0_, 128 * (t0_ + CB))
            nc.sync.dma_start(
                xb[:], xf16_d[nsl, :].rearrange("(t p) f -> p t f", p=128))
            for i in range(CB):
                tl = tl0 + i
                oh = oh_t[:, tl * n_graphs:(tl + 1) * n_graphs]
                nc.tensor.matmul(pool_ps[:], oh, xb[:, i, :],
                                 start=(tl == 0), stop=(tl == TH - 1))
    pooled_sb = sbpool.tile([n_graphs, F_DIM_], f32, tag="pooled")
    nc.vector.tensor_copy(pooled_sb[:], pool_pss[0][:])
    nc.vector.tensor_tensor(pooled_sb[:], pooled_sb[:], pool_pss[1][:],
                            op=Alu.add)
    nc.sync.dma_start(pooled_out[:], pooled_sb[:])


def _prep_weights(W1, b1, W2, b2):
    w2 = W2[:, 0].astype(np.float64)
    W1d = W1.astype(np.float64)
    v = W1d @ w2
    W1S = W1d * w2[None, :]
    pos = np.where(w2 >= 0)[0]
    neg = np.where(w2 < 0)[0]
    perm = np.concatenate([pos, neg])
    nP = len(pos)
    M = np.zeros((F_DIM, 258), np.float64)
    M[:, 0:256] = W1S[:, perm]
    M[:, 256] = v
    Mh, Ml = _split_bf16(M)
    return Mh, Ml, nP


def _numpy_fallback(x, batch, W1, b1, W2, b2):
    """Reference port in numpy for unexpected input structure."""
    N = x.shape[0]
    B = int(batch.max()) + 1 if batch.size else 0
    B = max(B, 1)
    scores = (np.maximum(x @ W1 + b1, 0.0) @ W2 + b2)[:, 0]
    counts = np.zeros(B, np.float32)
    np.add.at(counts, batch, 1.0)
    k = np.maximum(1, np.floor(RATIO * counts).astype(np.int64))
    order = np.lexsort((-scores, batch))
    b_sorted = batch[order]
    starts = np.cumsum(counts) - counts
    rank = np.arange(N) - starts[b_sorted]
    sel_sorted = rank < k[b_sorted]
    selected = np.zeros(N, bool)
    selected[order] = sel_sorted
    m = selected.astype(np.float32)
    xp = np.zeros((B, x.shape[1]), np.float32)
    np.add.at(xp, batch, x * m[:, None])
    sel_cnt = np.zeros(B, np.float32); np.add.at(sel_cnt, batch, m)
    sel_sum = np.zeros(B, np.float32); np.add.at(sel_sum, batch, scores * m)
    tot_sum = np.zeros(B, np.float32); np.add.at(tot_sum, batch, scores)
    uns_cnt = counts - sel_cnt
    sel_mean = sel_sum / np.maximum(sel_cnt, 1.0)
    uns_mean = (tot_sum - sel_sum) / np.maximum(uns_cnt, 1.0)
    pg = np.where(uns_cnt > 0, np.maximum(MARGIN - (sel_mean - uns_mean), 0.0), 0.0)
    loss = np.float32(pg.sum() / B * LOSS_W)
    return xp.astype(np.float32), loss, selected


_REFINE_SRC = r"""
import os, sys
os.environ["JAX_PLATFORMS"] = "cpu"
import numpy as np
import jax
jax.config.update("jax_platforms", "cpu")
d = np.load(sys.argv[1])
x = d["x"]; W1 = d["W1"]; b1 = d["b1"]; W2 = d["W2"]; b2 = d["b2"]
gidx = d["gidx"]; seg = int(d["seg"])
outs = []
for g in gidx:
    xs = x[g * seg:(g + 1) * seg]
    s = np.asarray((jax.nn.relu(xs @ W1 + b1) @ W2 + b2)[:, 0])
    outs.append(s)
np.savez(sys.argv[2], scores=np.stack(outs) if outs else np.zeros((0, seg)))
"""


def _refine_scores(x, W1, b1, W2, b2, gidx):
    """Recompute per-graph scores bit-exactly as the jax-cpu reference."""
    with tempfile.TemporaryDirectory() as td:
        inp = os.path.join(td, "in.npz")
        outp = os.path.join(td, "out.npz")
        script = os.path.join(td, "refine.py")
        np.savez(inp, x=x, W1=W1, b1=b1, W2=W2, b2=b2,
                 gidx=np.asarray(gidx, np.int64), seg=SEG)
        with open(script, "w") as f:
            f.write(_REFINE_SRC)
        env = dict(os.environ)
        env["JAX_PLATFORMS"] = "cpu"
        subprocess.run([sys.executable, script, inp, outp], check=True, env=env,
                       stdout=subprocess.DEVNULL, stderr=subprocess.DEVNULL)
        return np.load(outp)["scores"].astype(np.float32)


def kernel(x, batch, W1, b1, W2, b2):
    x = np.asarray(x); batch = np.asarray(batch)
    W1 = np.asarray(W1); b1 = np.asarray(b1)
    W2 = np.asarray(W2); b2 = np.asarray(b2)

    expected_batch = (np.arange(N_TOTAL, dtype=np.int64) // SEG).astype(batch.dtype)
    if (x.shape != (N_TOTAL, F_DIM) or batch.shape != (N_TOTAL,)
            or not np.array_equal(batch, expected_batch) or b1.any()):
        return _numpy_fallback(x, batch, W1, b1, W2, b2)

    from concourse.bass_utils import run_bass_kernel_spmd

    Mh, Ml, nP = _prep_weights(W1, b1, W2, b2)

    key = (N_PC, G_PC, nP)
    if key not in _prog_cache:
        _prog_cache[key] = _build_program(N_PC, G_PC, nP)
    nc = _prog_cache[key]

    x64 = x.astype(np.float64)
    xh, xl = _split_bf16(x64)
    xhT = np.ascontiguousarray(xh.T)
    xlT = np.ascontiguousarray(xl.T)
    xf16 = x.astype(np.float16)

    in_maps = []
    for c in range(N_CORES):
        nsl = slice(c * N_PC, (c + 1) * N_PC)
        in_maps.append({
            "xhT": np.ascontiguousarray(xhT[:, nsl]),
            "xlT": np.ascontiguousarray(xlT[:, nsl]),
            "xf16": xf16[nsl],
            "Mh": Mh,
            "Ml": Ml,
        })

    trace = bool(os.environ.get("KERNEL_TRACE"))
    res = None
    last_exc = None
    for attempt in range(4):
        try:
            res = run_bass_kernel_spmd(nc, in_maps,
                                       core_ids=list(range(N_CORES)),
                                       trace=trace)
            break
        except Exception as e:   # device wedge / transient exec failure
            last_exc = e
            import time as _time
            _time.sleep(20.0 * (attempt + 1))
    if res is None:
        sys.stderr.write("device run failed (%r); numpy fallback\n" % (last_exc,))
        return _numpy_fallback(x, batch, W1, b1, W2, b2)
    global LAST_RESULTS
    LAST_RESULTS = res

    pooled = np.concatenate([r["pooled_out"] for r in res.results], axis=0)
    scores = np.concatenate([r["scores_out"] for r in res.results], axis=0)
    maskf = np.concatenate([r["mask_out"] for r in res.results], axis=0)
    sums = np.concatenate([r["sums_out"] for r in res.results], axis=0)

    mask = maskf > 0.5                       # [B, 256]
    sel_sum = sums[:, 0].copy()
    tot_sum = sums[:, 1].copy()
    thr = sums[:, 2]                         # 128th largest
    below = sums[:, 3]                       # 129th largest
    margin = thr - below

    # graphs needing exact tie-refinement
    risky = np.where(margin < REFINE_EPS)[0]
    if len(risky):
        s_ref = _refine_scores(x, W1, b1, W2, b2, risky)
        for gi, g in enumerate(risky):
            s_g = s_ref[gi]
            order = np.argsort(-s_g, kind="stable")
            sel = np.zeros(SEG, bool)
            sel[order[:SEG // 2]] = True
            if not np.array_equal(sel, mask[g]):
                mask[g] = sel
                xg = x[g * SEG:(g + 1) * SEG]
                pooled[g] = (xg * sel[:, None].astype(np.float32)).sum(axis=0,
                                                                       dtype=np.float32)
            sel_sum[g] = np.float32(s_g[sel].sum(dtype=np.float32))
            tot_sum[g] = np.float32(s_g.sum(dtype=np.float32))

    kk = SEG // 2
    sel_mean = sel_sum / kk
    uns_mean = (tot_sum - sel_sum) / kk
    pg = np.maximum(MARGIN - (sel_mean - uns_mean), 0.0)
    loss = np.float32(pg.sum(dtype=np.float32) / B_TOTAL * LOSS_W)

    x_pooled = pooled.astype(np.float32)
    selected_mask = mask.reshape(-1)
    return x_pooled, loss, selected_mask


if __name__ == "__main__":
    import reference as ref
    inputs = ref.setup_inputs()
    out = kernel(**{k: np.asarray(v) for k, v in inputs.items()})
    print([o.shape if hasattr(o, "shape") else o for o in out])
